# revision 15
# baseline (speedup 1.0000x reference)
"""Trainium2 Bass kernel for nn_Classifier_6717328851414.

DEQ-style classifier:
  150 iterations of  z <- 0.5*z + 0.5*lrelu(conv2(lrelu(conv1(cat(z, img)))))
  conv1: 8->6 ch 5x5 pad 2; conv2: 6->5 ch 5x5 pad 2; 32x32 images; then a
  5->10 channel 32x32 valid conv producing logits (N,10,1,1).

Strategy: pure data parallel over batch N=512 -> 64 images per NeuronCore.

v2 formulation (vs banded-x baseline): the image's conv1 contribution is
constant across iterations, so it is precomputed on the host (C_img, with b1
folded in) and the per-iteration conv1 contracts only over the 5 z channels.
Both convs use y-window packed contractions:

  partitions = (y_local, channel)   free = (x_padded, n)  [36*64 = 2304]

ky folds densely into the banded stationary (window rows x output-row cols);
kx becomes 5 free-offset accumulation passes into PSUM. Two y-tiles
(y' 0:16 / 16:32) x 4 x-chunks of 8 give the matmul grid. The batch is split
into 2 independent streams of 32 images: stream A's epilogue+halo tail hides
under stream B's matmul phase, so the PE never idles (and never falls off the
~3us pstate ramp). x-chunks of 16 give FD = 16x*32n = 512 (a full PSUM bank):
conv1 = 5*2*2*2 = 40 MMs, conv2 = 40 MMs per iteration (fp32r, FD=512).

PSUM partition index == slab partition index by padding the stationary with
zero-weight dummy M-columns, so every per-iter epilogue (C_img add + Lrelu on
ACT/DVE, damped z update on DVE) is lane-aligned. The only lane-crossing ops
are 4 small halo exchanges per iteration (rows y=14,15 / 16,17 duplicated
between the two y-window slabs), done with partition-shifted SBUF->SBUF DMA,
chunked along x so they hide under the matmul stream.

Out-of-range y taps take zero weights in the stationary (no pad rows), and
each slab stores its 16 content rows FIRST so that PSUM partition 0 == slab
partition 0 (engines require 32-aligned partition bases; only DMA can shift).

Slab layouts (per core, per stream s, fp32; free = (xp, nl) = 36*32 = 1152):
  Z[s][0] [90,1152]: p=y*5+c, y in [0,18): content 0:80 (y 0:16),
                     halo 80:90 (y 16,17) <- Z[s][1][0:10]
  Z[s][1] [90,1152]: p=(y-16)*5+c for y in [16,32) (content 0:80),
                     80:90 = y 14,15 (halo) <- Z[s][0][70:80]
  H[s][0] [108,1152]: p=y*6+c, content 0:96, halo 96:108 <- H[s][1][0:12]
  H[s][1] [108,1152]: content 0:96 = y 16:32, halo 96:108 = y 14,15 <- H[s][0][84:96]
"""

import numpy as np

import concourse.bass as bass
import concourse.mybir as mybir
import concourse.tile as tile
from concourse.vector_clock import ScopedClock, VectorClock

ITERS = 150
SLOPE = 0.01
NCORES = 8
NTOT = 512
NPER = NTOT // NCORES  # 64
XP = 36  # padded x
NH = NPER // 2  # 32 images per stream
FREE = XP * NH  # 1152 per stream slab
F32 = mybir.dt.float32
F32R = mybir.dt.float32r
AF = mybir.ActivationFunctionType
OP = mybir.AluOpType


def _patched_drain_and_barrier(self, tick_clock, wait_clock):
    # Workaround: this walrus rejects >2 sync waits on one instruction
    # ("Too many sync wait commands"). Split the final drain's waits across
    # one SP nop per logical processor.
    gc = tick_clock.global_clock
    n = len(gc)
    for p in range(n):
        if gc[p] == 0:
            continue
        vc = VectorClock([gc[q] if q == p else 0 for q in range(n)])
        nop = self.nc.sync.nop(nofuse=True)
        wait_clock.add_sem_waits(nop.ins, ScopedClock({None: vc}))
    self.nc.sync.drain()
    self.nc.all_engine_barrier()
    assert self.sems is not None
    popped = self.nc._tile_sem_poison_stack.pop()
    assert popped is self._sem_poison
    self.nc.clear_and_free_semaphores(list(self.sems.allocated().values()))
    self.nc.all_engine_barrier()


tile.TileContext._drain_and_barrier = _patched_drain_and_barrier


def _split_excess_waits(nc, limit=1):
    """Walrus codegen rejects instructions with >2 sync waits (>1 for the
    self-loading fp32 matmul's LDWEIGHTS struct); hoist the excess onto
    same-engine NoOps placed immediately before."""
    for bb in nc.main_func.blocks:
        out = []
        changed = False
        for ins in bb.instructions:
            lim = limit
            si = ins.sync_info
            waits = list(si.on_wait) if (si is not None and si.on_wait) else []
            if len(waits) > lim:
                extra, keep = waits[:-lim], waits[-lim:]
                for i0 in range(0, len(extra), limit):
                    nop = mybir.InstNoOp(
                        name=nc.get_next_instruction_name(),
                        engine=ins.engine,
                        ins=[],
                        outs=[],
                        sync_info=mybir.SyncInfo(
                            on_wait=extra[i0 : i0 + limit], on_update=[]
                        ),
                    )
                    out.append(nop)
                si.on_wait = keep
                changed = True
            out.append(ins)
        if changed:
            bb.instructions = out
    return nc


def build_nc(iters=ITERS, unroll=15):
    nc = bass.Bass()

    w1s_p = nc.declare_dram_parameter("w1s", [90, 960], F32R, isOutput=False)
    w2s_p = nc.declare_dram_parameter("w2s", [108, 800], F32R, isOutput=False)
    whs_p = nc.declare_dram_parameter("whs", [90, 640], F32R, isOutput=False)
    cimg_p = nc.declare_dram_parameter("cimg", [96, 4096], F32, isOutput=False)
    bias_p = nc.declare_dram_parameter("biasv", [128, 2], F32, isOutput=False)
    out_p = nc.declare_dram_parameter("out", [10, NPER], F32, isOutput=True)

    with tile.TileContext(nc) as tc:
        with (
            tc.tile_pool(name="const", bufs=1) as cpool,
            tc.tile_pool(name="state", bufs=1) as spool,
            tc.tile_pool(name="psum", bufs=8, space="PSUM") as ppool,
            tc.tile_pool(name="stage", bufs=4) as vpool,
        ):
            w1s = cpool.tile([90, 960], F32R, tag="w1s")
            w2s = cpool.tile([108, 800], F32R, tag="w2s")
            whs = cpool.tile([90, 640], F32R, tag="whs")
            cimg = cpool.tile([96, 4096], F32, tag="cimg")
            biasv = cpool.tile([128, 2], F32, tag="biasv")
            nc.sync.dma_start(w1s[:], w1s_p[:])
            nc.sync.dma_start(w2s[:], w2s_p[:])
            nc.sync.dma_start(whs[:], whs_p[:])
            nc.sync.dma_start(cimg[:], cimg_p[:])
            nc.sync.dma_start(biasv[:], bias_p[:])

            Z = [
                [spool.tile([90, FREE], F32, tag=f"Z{s}{t}", name=f"Z{s}{t}") for t in range(2)]
                for s in range(2)
            ]
            H = [
                [spool.tile([108, FREE], F32, tag=f"H{s}{t}", name=f"H{s}{t}") for t in range(2)]
                for s in range(2)
            ]
            for s in range(2):
                for t in range(2):
                    nc.gpsimd.memset(Z[s][t][:], 0.0)
                    nc.gpsimd.memset(H[s][t][:], 0.0)

            def conv1_mms(s):
                ps1 = {}
                for t in range(2):
                    for f in range(2):
                        ps1[(t, f)] = ppool.tile(
                            [96, 512], F32, tag="ps", name=f"ps1_{s}_{t}_{f}"
                        )
                # f outer / kx inner: PSUM groups complete spread through the
                # phase, so epilogue chains pipeline instead of bunching at the
                # end.
                for t in range(2):
                    for f in range(2):
                        for kx in range(5):
                            col = (t * 5 + kx) * 96
                            off = (f * 16 + kx) * NH
                            nc.tensor.matmul(
                                ps1[(t, f)][:],
                                w1s[:, col : col + 96],
                                Z[s][t][:, off : off + 512].bitcast(F32R),
                                start=(kx == 0),
                                stop=(kx == 4),
                            )
                return ps1

            def conv1_epi(s, ps1):
                for t in range(2):
                    for f in range(2):
                        tmp = vpool.tile([96, 512], F32, tag="tmp", name=f"tmp_{s}_{t}_{f}")
                        nc.vector.tensor_tensor(
                            tmp[:],
                            ps1[(t, f)][:],
                            cimg[:, s * 2048 + t * 1024 + f * 512 : s * 2048 + t * 1024 + (f + 1) * 512],
                            OP.add,
                        )
                        dcol = (f * 16 + 2) * NH
                        nc.scalar.activation(
                            H[s][t][0:96, dcol : dcol + 512].bitcast(F32R),
                            tmp[:],
                            AF.Lrelu,
                            bias=0.0,
                            scale=1.0,
                            alpha=SLOPE,
                        )
                # h1 halo exchange (shifted partitions -> DMA)
                nc.sync.dma_start(
                    H[s][0][96:108, :].bitcast(F32R), H[s][1][0:12, :].bitcast(F32R)
                )
                nc.sync.dma_start(
                    H[s][1][96:108, :].bitcast(F32R), H[s][0][84:96, :].bitcast(F32R)
                )

            def conv2_mms(s):
                ps2 = {}
                for t in range(2):
                    for f in range(2):
                        ps2[(t, f)] = ppool.tile(
                            [80, 512], F32, tag="ps", name=f"ps2_{s}_{t}_{f}"
                        )
                for t in range(2):
                    for f in range(2):
                        for kx in range(5):
                            col = (t * 5 + kx) * 80
                            off = (f * 16 + kx) * NH
                            nc.tensor.matmul(
                                ps2[(t, f)][:],
                                w2s[:, col : col + 80],
                                H[s][t][:, off : off + 512].bitcast(F32R),
                                start=(kx == 0),
                                stop=(kx == 4),
                            )
                return ps2

            def conv2_epi(s, ps2):
                for t in range(2):
                    for f in range(2):
                        v = vpool.tile([80, 512], F32, tag="v", name=f"v_{s}_{t}_{f}")
                        nc.scalar.activation(
                            v[:],
                            ps2[(t, f)][:],
                            AF.Lrelu,
                            bias=biasv[0:80, 0:1],
                            scale=0.5,
                            alpha=SLOPE,
                        )
                        dcol = (f * 16 + 2) * NH
                        nc.vector.scalar_tensor_tensor(
                            Z[s][t][0:80, dcol : dcol + 512].bitcast(F32R),
                            Z[s][t][0:80, dcol : dcol + 512],
                            0.5,
                            v[:],
                            OP.mult,
                            OP.add,
                        )
                # z halo exchange
                nc.sync.dma_start(
                    Z[s][0][80:90, :].bitcast(F32R), Z[s][1][0:10, :].bitcast(F32R)
                )
                nc.sync.dma_start(
                    Z[s][1][80:90, :].bitcast(F32R), Z[s][0][70:80, :].bitcast(F32R)
                )

            def one_iter():
                # stream-pipelined: s0's epilogues/halos hide under s1's MMs
                # and vice versa.
                p10 = conv1_mms(0)
                p11 = conv1_mms(1)
                conv1_epi(0, p10)
                p20 = conv2_mms(0)
                conv1_epi(1, p11)
                p21 = conv2_mms(1)
                conv2_epi(0, p20)
                conv2_epi(1, p21)

            trips, rem = divmod(iters, unroll)
            if trips > 0:
                with tc.For_i(0, trips, 1):
                    for _ in range(unroll):
                        one_iter()
            for _ in range(rem):
                one_iter()

            # ---- head: logits[k, nl] = sum_{c,y,x} wh * z + bh
            psh = ppool.tile([10, NPER], F32, tag="ps")
            for s in range(2):
                k = 0
                for t in range(2):
                    for x in range(32):
                        nc.tensor.matmul(
                            psh[:, s * NH : (s + 1) * NH],
                            whs[:, (t * 32 + x) * 10 : (t * 32 + x + 1) * 10],
                            Z[s][t][:, (x + 2) * NH : (x + 3) * NH].bitcast(F32R),
                            start=(k == 0),
                            stop=(k == 63),
                        )
                        k += 1
            out_sb = vpool.tile([10, NPER], F32, tag="osb")
            nc.scalar.activation(
                out_sb[:], psh[:], AF.Identity, bias=biasv[0:10, 1:2], scale=1.0
            )
            nc.sync.dma_start(out_p[:], out_sb[:])

    _split_excess_waits(nc)
    return nc


def _np_conv_same(x, w, b):
    # 5x5 pad-2 cross-correlation via 25 shifted tensordots: x [N,C,32,32]
    N, C, Hh, Ww = x.shape
    O = w.shape[0]
    xp = np.zeros((N, C, Hh + 4, Ww + 4), np.float32)
    xp[:, :, 2 : 2 + Hh, 2 : 2 + Ww] = x
    out = np.zeros((O, N, Hh, Ww), np.float32)
    for ky in range(5):
        for kx in range(5):
            out += np.tensordot(
                w[:, :, ky, kx], xp[:, :, ky : ky + Hh, kx : kx + Ww], axes=([1], [1])
            )
    return out.transpose(1, 0, 2, 3) + b[None, :, None, None]


def pack_inputs(image, w1, b1, w2, b2, wh, bh):
    """Host-side transforms; returns (shared dict, per-core cimg list)."""
    image = np.asarray(image, dtype=np.float32)
    w1 = np.asarray(w1, dtype=np.float32)
    b1 = np.asarray(b1, dtype=np.float32)
    w2 = np.asarray(w2, dtype=np.float32)
    b2 = np.asarray(b2, dtype=np.float32)
    wh = np.asarray(wh, dtype=np.float32)
    bh = np.asarray(bh, dtype=np.float32)

    # slab row maps: list of input y per row-block (t=1 stores content first,
    # then the two halo rows y=14,15)
    yins = [list(range(0, 18)), list(range(16, 32)) + [14, 15]]

    w1s = np.zeros((2, 5, 90, 96), np.float32)
    w2s = np.zeros((2, 5, 108, 80), np.float32)
    for t in range(2):
        ypbase = 0 if t == 0 else 16
        for ri, yin in enumerate(yins[t]):
            for yp in range(16):
                ky = yin - (ypbase + yp) + 2
                if not (0 <= ky < 5):
                    continue
                for kx in range(5):
                    for ci in range(5):
                        for co in range(6):
                            w1s[t, kx, ri * 5 + ci, yp * 6 + co] = w1[co, ci, ky, kx]
                    for ci in range(6):
                        for co in range(5):
                            w2s[t, kx, ri * 6 + ci, yp * 5 + co] = w2[co, ci, ky, kx]
    w1s = w1s.transpose(2, 0, 1, 3).reshape(90, 960)
    w2s = w2s.transpose(2, 0, 1, 3).reshape(108, 800)

    # head stationaries [90, (t,x,k)]: halo rows (80:90) stay zero
    whs = np.zeros((2, 90, 320), np.float32)
    for t in range(2):
        for y in range(16):
            yg = y if t == 0 else 16 + y
            for c in range(5):
                whs[t, y * 5 + c] = wh[:, c, yg, :].T.reshape(320)  # (x,k)
    whs = whs.transpose(1, 0, 2).reshape(90, 640)

    biasv = np.zeros((128, 2), np.float32)
    for p in range(80):
        biasv[p, 0] = 0.5 * b2[p % 5]
    biasv[0:10, 1] = bh

    # C_img: conv1 applied to image channels only, + b1 (constant across iters)
    cfull = _np_conv_same(image, w1[:, 5:8], b1)  # [512, 6, 32, 32]
    cimgs = []
    for c in range(NCORES):
        sl = cfull[c * NPER : (c + 1) * NPER]  # [64, 6, 32, 32]
        arr = sl.transpose(2, 1, 3, 0)  # (y, co, x, n)
        ci = np.zeros((96, 4096), np.float32)
        for s in range(2):
            a = arr[:, :, :, s * 32 : (s + 1) * 32]  # (y, co, x, nl)
            ci[:, s * 2048 : s * 2048 + 1024] = a[0:16].reshape(96, 1024)
            ci[:, s * 2048 + 1024 : (s + 1) * 2048] = a[16:32].reshape(96, 1024)
        cimgs.append(ci)

    shared = {"w1s": w1s, "w2s": w2s, "whs": whs, "biasv": biasv}
    return shared, cimgs


_NC_CACHE = {}


def _get_nc(iters, unroll=15):
    key = (iters, unroll)
    if key not in _NC_CACHE:
        _NC_CACHE[key] = build_nc(iters, unroll)
    return _NC_CACHE[key]


def kernel(image, w1, b1, w2, b2, wh, bh, _iters=ITERS, _unroll=15):
    from concourse.bass_utils import run_bass_kernel_spmd

    shared, cimgs = pack_inputs(image, w1, b1, w2, b2, wh, bh)
    in_maps = [dict(shared, cimg=cimgs[c]) for c in range(NCORES)]
    nc = _get_nc(_iters, _unroll)
    res = run_bass_kernel_spmd(nc, in_maps, list(range(NCORES)))
    outs = []
    for c in range(NCORES):
        o = res.results[c]["out"]  # [10, 64]
        outs.append(o.T)  # [64, 10]
    logits = np.concatenate(outs, axis=0).astype(np.float32)  # [512, 10]
    return logits.reshape(NTOT, 10, 1, 1)


# revision 17
# speedup vs baseline: 1.0986x; 1.0986x over previous
"""Trainium2 Bass kernel for nn_Classifier_6717328851414.

DEQ-style classifier:
  150 iterations of  z <- 0.5*z + 0.5*lrelu(conv2(lrelu(conv1(cat(z, img)))))
  conv1: 8->6 ch 5x5 pad 2; conv2: 6->5 ch 5x5 pad 2; 32x32 images; then a
  5->10 channel 32x32 valid conv producing logits (N,10,1,1).

Strategy: pure data parallel over batch N=512 -> 64 images per NeuronCore.

v2 formulation (vs banded-x baseline): the image's conv1 contribution is
constant across iterations, so it is precomputed on the host (C_img, with b1
folded in) and the per-iteration conv1 contracts only over the 5 z channels.
Both convs use y-window packed contractions:

  partitions = (y_local, channel)   free = (x_padded, n)  [36*64 = 2304]

ky folds densely into the banded stationary (window rows x output-row cols);
kx becomes 5 free-offset accumulation passes into PSUM. Two y-tiles
(y' 0:16 / 16:32) x 4 x-chunks of 8 give the matmul grid. The batch is split
into 2 independent streams of 32 images (FD = 8x*32n = 256, still full fp32r
rate): stream A's epilogue+halo tail hides under stream B's matmul phase, so
the PE never idles (and never falls off the ~3us pstate ramp).
conv1 = 5*2*4*2 = 80 MMs, conv2 = 80 MMs per iteration (fp32r, FD=256).

PSUM partition index == slab partition index by padding the stationary with
zero-weight dummy M-columns, so every per-iter epilogue (C_img add + Lrelu on
ACT/DVE, damped z update on DVE) is lane-aligned. The only lane-crossing ops
are 4 small halo exchanges per iteration (rows y=14,15 / 16,17 duplicated
between the two y-window slabs), done with partition-shifted SBUF->SBUF DMA,
chunked along x so they hide under the matmul stream.

Out-of-range y taps take zero weights in the stationary (no pad rows), and
each slab stores its 16 content rows FIRST so that PSUM partition 0 == slab
partition 0 (engines require 32-aligned partition bases; only DMA can shift).

Slab layouts (per core, per stream s, fp32; free = (xp, nl) = 36*32 = 1152):
  Z[s][0] [90,1152]: p=y*5+c, y in [0,18): content 0:80 (y 0:16),
                     halo 80:90 (y 16,17) <- Z[s][1][0:10]
  Z[s][1] [90,1152]: p=(y-16)*5+c for y in [16,32) (content 0:80),
                     80:90 = y 14,15 (halo) <- Z[s][0][70:80]
  H[s][0] [108,1152]: p=y*6+c, content 0:96, halo 96:108 <- H[s][1][0:12]
  H[s][1] [108,1152]: content 0:96 = y 16:32, halo 96:108 = y 14,15 <- H[s][0][84:96]
"""

import numpy as np

import concourse.bass as bass
import concourse.mybir as mybir
import concourse.tile as tile
from concourse.vector_clock import ScopedClock, VectorClock

ITERS = 150
SLOPE = 0.01
NCORES = 8
NTOT = 512
NPER = NTOT // NCORES  # 64
XP = 36  # padded x
NH = NPER // 2  # 32 images per stream
FREE = XP * NH  # 1152 per stream slab
F32 = mybir.dt.float32
F32R = mybir.dt.float32r
AF = mybir.ActivationFunctionType
OP = mybir.AluOpType


def _patched_drain_and_barrier(self, tick_clock, wait_clock):
    # Workaround: this walrus rejects >2 sync waits on one instruction
    # ("Too many sync wait commands"). Split the final drain's waits across
    # one SP nop per logical processor.
    gc = tick_clock.global_clock
    n = len(gc)
    for p in range(n):
        if gc[p] == 0:
            continue
        vc = VectorClock([gc[q] if q == p else 0 for q in range(n)])
        nop = self.nc.sync.nop(nofuse=True)
        wait_clock.add_sem_waits(nop.ins, ScopedClock({None: vc}))
    self.nc.sync.drain()
    self.nc.all_engine_barrier()
    assert self.sems is not None
    popped = self.nc._tile_sem_poison_stack.pop()
    assert popped is self._sem_poison
    self.nc.clear_and_free_semaphores(list(self.sems.allocated().values()))
    self.nc.all_engine_barrier()


tile.TileContext._drain_and_barrier = _patched_drain_and_barrier


def _split_excess_waits(nc, limit=1):
    """Walrus codegen rejects instructions with >2 sync waits (>1 for the
    self-loading fp32 matmul's LDWEIGHTS struct); hoist the excess onto
    same-engine NoOps placed immediately before."""
    for bb in nc.main_func.blocks:
        out = []
        changed = False
        for ins in bb.instructions:
            lim = limit
            si = ins.sync_info
            waits = list(si.on_wait) if (si is not None and si.on_wait) else []
            if len(waits) > lim:
                extra, keep = waits[:-lim], waits[-lim:]
                for i0 in range(0, len(extra), limit):
                    nop = mybir.InstNoOp(
                        name=nc.get_next_instruction_name(),
                        engine=ins.engine,
                        ins=[],
                        outs=[],
                        sync_info=mybir.SyncInfo(
                            on_wait=extra[i0 : i0 + limit], on_update=[]
                        ),
                    )
                    out.append(nop)
                si.on_wait = keep
                changed = True
            out.append(ins)
        if changed:
            bb.instructions = out
    return nc


def build_nc(iters=ITERS, unroll=15):
    nc = bass.Bass()

    w1s_p = nc.declare_dram_parameter("w1s", [90, 960], F32R, isOutput=False)
    w2s_p = nc.declare_dram_parameter("w2s", [108, 800], F32R, isOutput=False)
    whs_p = nc.declare_dram_parameter("whs", [90, 640], F32R, isOutput=False)
    cimg_p = nc.declare_dram_parameter("cimg", [96, 4096], F32, isOutput=False)
    bias_p = nc.declare_dram_parameter("biasv", [128, 2], F32, isOutput=False)
    out_p = nc.declare_dram_parameter("out", [10, NPER], F32, isOutput=True)

    with tile.TileContext(nc) as tc:
        with (
            tc.tile_pool(name="const", bufs=1) as cpool,
            tc.tile_pool(name="state", bufs=1) as spool,
            tc.tile_pool(name="psum", bufs=8, space="PSUM") as ppool,
            tc.tile_pool(name="stage", bufs=4) as vpool,
        ):
            w1s = cpool.tile([90, 960], F32R, tag="w1s")
            w2s = cpool.tile([108, 800], F32R, tag="w2s")
            whs = cpool.tile([90, 640], F32R, tag="whs")
            cimg = cpool.tile([96, 4096], F32, tag="cimg")
            biasv = cpool.tile([128, 2], F32, tag="biasv")
            nc.sync.dma_start(w1s[:], w1s_p[:])
            nc.sync.dma_start(w2s[:], w2s_p[:])
            nc.sync.dma_start(whs[:], whs_p[:])
            nc.sync.dma_start(cimg[:], cimg_p[:])
            nc.sync.dma_start(biasv[:], bias_p[:])

            Z = [
                [spool.tile([90, FREE], F32, tag=f"Z{s}{t}", name=f"Z{s}{t}") for t in range(2)]
                for s in range(2)
            ]
            H = [
                [spool.tile([108, FREE], F32, tag=f"H{s}{t}", name=f"H{s}{t}") for t in range(2)]
                for s in range(2)
            ]
            for s in range(2):
                for t in range(2):
                    nc.gpsimd.memset(Z[s][t][:], 0.0)
                    nc.gpsimd.memset(H[s][t][:], 0.0)

            def conv1_mms(s):
                ps1 = {}
                for t in range(2):
                    for f in range(4):
                        ps1[(t, f)] = ppool.tile(
                            [96, 256], F32, tag="ps", name=f"ps1_{s}_{t}_{f}"
                        )
                # f outer / kx inner: PSUM groups complete spread through the
                # phase, so epilogue chains pipeline instead of bunching at the
                # end.
                for t in range(2):
                    for f in range(4):
                        for kx in range(5):
                            col = (t * 5 + kx) * 96
                            off = (f * 8 + kx) * NH
                            nc.tensor.matmul(
                                ps1[(t, f)][:],
                                w1s[:, col : col + 96],
                                Z[s][t][:, off : off + 256].bitcast(F32R),
                                start=(kx == 0),
                                stop=(kx == 4),
                            )
                return ps1

            def conv1_epi(s, ps1):
                for t in range(2):
                    for f in range(4):
                        # C_img add in-place in PSUM (DVE R+W), then ACT lrelu
                        # reads PSUM directly (faster than SBUF) into the slab.
                        nc.vector.tensor_tensor(
                            ps1[(t, f)][:],
                            ps1[(t, f)][:],
                            cimg[:, s * 2048 + t * 1024 + f * 256 : s * 2048 + t * 1024 + (f + 1) * 256],
                            OP.add,
                        )
                        dcol = (f * 8 + 2) * NH
                        nc.scalar.activation(
                            H[s][t][0:96, dcol : dcol + 256].bitcast(F32R),
                            ps1[(t, f)][:],
                            AF.Lrelu,
                            bias=0.0,
                            scale=1.0,
                            alpha=SLOPE,
                        )
                # h1 halo exchange (shifted partitions -> DMA)
                nc.sync.dma_start(
                    H[s][0][96:108, :].bitcast(F32R), H[s][1][0:12, :].bitcast(F32R)
                )
                nc.sync.dma_start(
                    H[s][1][96:108, :].bitcast(F32R), H[s][0][84:96, :].bitcast(F32R)
                )

            def conv2_mms(s):
                ps2 = {}
                for t in range(2):
                    for f in range(4):
                        ps2[(t, f)] = ppool.tile(
                            [80, 256], F32, tag="ps", name=f"ps2_{s}_{t}_{f}"
                        )
                for t in range(2):
                    for f in range(4):
                        for kx in range(5):
                            col = (t * 5 + kx) * 80
                            off = (f * 8 + kx) * NH
                            nc.tensor.matmul(
                                ps2[(t, f)][:],
                                w2s[:, col : col + 80],
                                H[s][t][:, off : off + 256].bitcast(F32R),
                                start=(kx == 0),
                                stop=(kx == 4),
                            )
                return ps2

            def conv2_epi(s, ps2):
                for t in range(2):
                    for f in range(4):
                        v = vpool.tile([80, 256], F32, tag="v", name=f"v_{s}_{t}_{f}")
                        nc.scalar.activation(
                            v[:],
                            ps2[(t, f)][:],
                            AF.Lrelu,
                            bias=biasv[0:80, 0:1],
                            scale=0.5,
                            alpha=SLOPE,
                        )
                        dcol = (f * 8 + 2) * NH
                        nc.vector.scalar_tensor_tensor(
                            Z[s][t][0:80, dcol : dcol + 256].bitcast(F32R),
                            Z[s][t][0:80, dcol : dcol + 256],
                            0.5,
                            v[:],
                            OP.mult,
                            OP.add,
                        )
                # z halo exchange (ACT DGE queue: decoupled from H-halo FIFO)
                nc.scalar.dma_start(
                    Z[s][0][80:90, :].bitcast(F32R), Z[s][1][0:10, :].bitcast(F32R)
                )
                nc.scalar.dma_start(
                    Z[s][1][80:90, :].bitcast(F32R), Z[s][0][70:80, :].bitcast(F32R)
                )

            def one_iter():
                # stream-pipelined: s0's epilogues/halos hide under s1's MMs
                # and vice versa.
                p10 = conv1_mms(0)
                p11 = conv1_mms(1)
                conv1_epi(0, p10)
                p20 = conv2_mms(0)
                conv1_epi(1, p11)
                p21 = conv2_mms(1)
                conv2_epi(0, p20)
                conv2_epi(1, p21)

            trips, rem = divmod(iters, unroll)
            if trips > 0:
                with tc.For_i(0, trips, 1):
                    for _ in range(unroll):
                        one_iter()
            for _ in range(rem):
                one_iter()

            # ---- head: logits[k, nl] = sum_{c,y,x} wh * z + bh
            psh = ppool.tile([10, NPER], F32, tag="ps")
            for s in range(2):
                k = 0
                for t in range(2):
                    for x in range(32):
                        nc.tensor.matmul(
                            psh[:, s * NH : (s + 1) * NH],
                            whs[:, (t * 32 + x) * 10 : (t * 32 + x + 1) * 10],
                            Z[s][t][:, (x + 2) * NH : (x + 3) * NH].bitcast(F32R),
                            start=(k == 0),
                            stop=(k == 63),
                        )
                        k += 1
            out_sb = vpool.tile([10, NPER], F32, tag="osb")
            nc.scalar.activation(
                out_sb[:], psh[:], AF.Identity, bias=biasv[0:10, 1:2], scale=1.0
            )
            nc.sync.dma_start(out_p[:], out_sb[:])

    _split_excess_waits(nc)
    return nc


def _np_conv_same(x, w, b):
    # 5x5 pad-2 cross-correlation via 25 shifted tensordots: x [N,C,32,32]
    N, C, Hh, Ww = x.shape
    O = w.shape[0]
    xp = np.zeros((N, C, Hh + 4, Ww + 4), np.float32)
    xp[:, :, 2 : 2 + Hh, 2 : 2 + Ww] = x
    out = np.zeros((O, N, Hh, Ww), np.float32)
    for ky in range(5):
        for kx in range(5):
            out += np.tensordot(
                w[:, :, ky, kx], xp[:, :, ky : ky + Hh, kx : kx + Ww], axes=([1], [1])
            )
    return out.transpose(1, 0, 2, 3) + b[None, :, None, None]


def pack_inputs(image, w1, b1, w2, b2, wh, bh):
    """Host-side transforms; returns (shared dict, per-core cimg list)."""
    image = np.asarray(image, dtype=np.float32)
    w1 = np.asarray(w1, dtype=np.float32)
    b1 = np.asarray(b1, dtype=np.float32)
    w2 = np.asarray(w2, dtype=np.float32)
    b2 = np.asarray(b2, dtype=np.float32)
    wh = np.asarray(wh, dtype=np.float32)
    bh = np.asarray(bh, dtype=np.float32)

    # slab row maps: list of input y per row-block (t=1 stores content first,
    # then the two halo rows y=14,15)
    yins = [list(range(0, 18)), list(range(16, 32)) + [14, 15]]

    w1s = np.zeros((2, 5, 90, 96), np.float32)
    w2s = np.zeros((2, 5, 108, 80), np.float32)
    for t in range(2):
        ypbase = 0 if t == 0 else 16
        for ri, yin in enumerate(yins[t]):
            for yp in range(16):
                ky = yin - (ypbase + yp) + 2
                if not (0 <= ky < 5):
                    continue
                for kx in range(5):
                    for ci in range(5):
                        for co in range(6):
                            w1s[t, kx, ri * 5 + ci, yp * 6 + co] = w1[co, ci, ky, kx]
                    for ci in range(6):
                        for co in range(5):
                            w2s[t, kx, ri * 6 + ci, yp * 5 + co] = w2[co, ci, ky, kx]
    w1s = w1s.transpose(2, 0, 1, 3).reshape(90, 960)
    w2s = w2s.transpose(2, 0, 1, 3).reshape(108, 800)

    # head stationaries [90, (t,x,k)]: halo rows (80:90) stay zero
    whs = np.zeros((2, 90, 320), np.float32)
    for t in range(2):
        for y in range(16):
            yg = y if t == 0 else 16 + y
            for c in range(5):
                whs[t, y * 5 + c] = wh[:, c, yg, :].T.reshape(320)  # (x,k)
    whs = whs.transpose(1, 0, 2).reshape(90, 640)

    biasv = np.zeros((128, 2), np.float32)
    for p in range(80):
        biasv[p, 0] = 0.5 * b2[p % 5]
    biasv[0:10, 1] = bh

    # C_img: conv1 applied to image channels only, + b1 (constant across iters)
    cfull = _np_conv_same(image, w1[:, 5:8], b1)  # [512, 6, 32, 32]
    cimgs = []
    for c in range(NCORES):
        sl = cfull[c * NPER : (c + 1) * NPER]  # [64, 6, 32, 32]
        arr = sl.transpose(2, 1, 3, 0)  # (y, co, x, n)
        ci = np.zeros((96, 4096), np.float32)
        for s in range(2):
            a = arr[:, :, :, s * 32 : (s + 1) * 32]  # (y, co, x, nl)
            ci[:, s * 2048 : s * 2048 + 1024] = a[0:16].reshape(96, 1024)
            ci[:, s * 2048 + 1024 : (s + 1) * 2048] = a[16:32].reshape(96, 1024)
        cimgs.append(ci)

    shared = {"w1s": w1s, "w2s": w2s, "whs": whs, "biasv": biasv}
    return shared, cimgs


_NC_CACHE = {}


def _get_nc(iters, unroll=15):
    key = (iters, unroll)
    if key not in _NC_CACHE:
        _NC_CACHE[key] = build_nc(iters, unroll)
    return _NC_CACHE[key]


def kernel(image, w1, b1, w2, b2, wh, bh, _iters=ITERS, _unroll=15):
    from concourse.bass_utils import run_bass_kernel_spmd

    shared, cimgs = pack_inputs(image, w1, b1, w2, b2, wh, bh)
    in_maps = [dict(shared, cimg=cimgs[c]) for c in range(NCORES)]
    nc = _get_nc(_iters, _unroll)
    res = run_bass_kernel_spmd(nc, in_maps, list(range(NCORES)))
    outs = []
    for c in range(NCORES):
        o = res.results[c]["out"]  # [10, 64]
        outs.append(o.T)  # [64, 10]
    logits = np.concatenate(outs, axis=0).astype(np.float32)  # [512, 10]
    return logits.reshape(NTOT, 10, 1, 1)
